# revision 38
# baseline (speedup 1.0000x reference)
"""Trainium2 Bass kernel for nn_AutoencoderHybrid (12-qubit QAE hybrid).

Math: the circuit measures Z on wires 0..3 only; the CNOT chain propagates
forward only, so each observable pulled back is supported on wires 0..4
(5-qubit truncation, error ~3e-4). With the product input state the latent is
a real quadratic form lat_w(b) = r_b^T S_w r_b on the 32-dim product vector
r_b = kron_{j=0..4} [cos(x_j/2), sin(x_j/2)].

Device algorithm (per core, 1024 rows laid out b = 8p + q, fp16 datapath):
  eigendecompose S_w = sum_m lam u u^T on host; fold sqrt|lam| into
  U4[k, 32w+m] and sign(lam)*W1 into SW[32w+m, c].  Then per 4-group half:
    cs  = sin/cos(x/2) via ACT Sin (cos = Sin(x/2 + pi/2); the table arg
          reaches ~3.9 in the N(0,1) tail where Sin is still ~2e-3 accurate)
    r   = kron tree (DVE / GPSIMD, fp16)
    rT  = PE transpose (fp16 PSUM) + copy to SBUF
    G   = U4blk_g.T @ rT per group   (PE, 4 matmuls -> one PSUM bank)
    P   = G^2                        (ACT Square / DVE copy+mult, fp16 out)
    hT  = SWblk_g.T @ P_g + b1       (PE, 4 matmuls + K=1 bias prefill)
    y4  = relu(hT).T @ W2blk + b2    (relu on DVE/ACT, PE matmul)
  i.e. the latent reduction and the first MLP layer collapse into matmuls —
  no DVE reduce, no latent transpose.  Constants ship as TWO fp16 DMAs
  (critical blob first so the depth-4 PE wait queue is never plugged), and
  12 dependency-free warm-up matmuls hold the PE p-state at full clock
  through the input-DMA wait.  Engine assignments in CFG are the winner of
  a TimelineSim sweep.  14312ns baseline -> 11673ns.
"""
import math
import numpy as np

N5 = 5
NLAYERS = 2
LATENT = 4
B = 8192
NCORES = 8
BLOC = B // NCORES  # 1024

# ----------------------------------------------------------------------------
# Host-side constant construction (pure numpy)
# ----------------------------------------------------------------------------


def _rot(phi, theta, omega):
    c, s = np.cos(theta / 2), np.sin(theta / 2)
    ep = np.exp(-0.5j * (phi + omega))
    em = np.exp(-0.5j * (phi - omega))
    return np.array([[ep * c, -np.conj(em) * s], [em * s, np.conj(ep) * c]],
                    dtype=np.complex128)


def _build_S(q_params):
    """(4, 32, 32) real symmetric: latent_w = r^T S_w r."""
    qp = np.asarray(q_params, np.float64)
    dim = 2 ** N5
    eye2 = np.eye(2)

    def kron_at(U, wire):
        M = np.array([[1.0]])
        for j in range(N5):
            M = np.kron(M, U if j == wire else eye2)
        return M

    def cnot_mat(c, t):
        M = np.zeros((dim, dim))
        for z in range(dim):
            bits = [(z >> (N5 - 1 - j)) & 1 for j in range(N5)]
            if bits[c] == 1:
                bits[t] ^= 1
            z2 = 0
            for b in bits:
                z2 = (z2 << 1) | b
            M[z2, z] = 1.0
        return M

    V = np.eye(dim, dtype=np.complex128)
    for l in range(NLAYERS):
        for i in range(N5):
            V = kron_at(_rot(*qp[l, i]), i) @ V
        for i in range(N5 - 1):
            V = cnot_mat(i, i + 1) @ V

    pc = np.array([bin(z).count("1") for z in range(dim)])
    D = np.diag((-1j) ** pc)
    VD = V @ D
    Ss = []
    for w in range(LATENT):
        zdiag = np.array([1.0 if ((z >> (N5 - 1 - w)) & 1) == 0 else -1.0
                          for z in range(dim)])
        O = VD.conj().T @ (zdiag[:, None] * VD)
        Ss.append(np.real(O))
    return np.stack(Ss)


CB1 = 816   # critical const blob: ident, U4blk, b1, b2
CB2 = 560   # late const blob: SWblk, w2blk

# Engine-assignment configuration (winner of a TimelineSim sweep).
CFG = {
    "rt_copy": ["act", "dve"],          # rT PSUM->SBUF copy per half
    "sq": [["act", "act"], ["dve", "act"]],  # square engine per (half,chunk)
    "relu": ["dve", "act"],             # relu per half
    "y_copy": ["dve", "act"],           # y4 PSUM->SBUF copy per half
    "split_out": False,                 # two output DMAs vs one
    "pe_fillers": 12,                   # dummy matmuls to hold PE pstate up
    "kron": ["dve", "gpsimd"],          # kron tree engine per half
    "tr_edge": False,                   # force transpose h0 before h1
    "sq_edge": False,                   # force square h0A before h1A on ACT
}


def _host_consts(q_params, W1, b1, W2, b2):
    S = _build_S(q_params)                       # (4, 32, 32) float64
    W1 = np.asarray(W1, np.float64)              # (32, 4)
    b1 = np.asarray(b1, np.float64)              # (32,)
    W2 = np.asarray(W2, np.float64)              # (12, 32)
    b2 = np.asarray(b2, np.float64)              # (12,)

    U4 = np.zeros((32, 128))                     # [k, 32w+m] = sqrt|lam| u
    SW = np.zeros((128, 32))                     # [32w+m, c] = sign(lam) W1
    for w in range(LATENT):
        lam, U = np.linalg.eigh(S[w])            # S_w = U diag(lam) U^T
        U4[:, 32 * w:32 * w + 32] = U * np.sqrt(np.abs(lam))[None, :]
        SW[32 * w:32 * w + 32, :] = np.sign(lam)[:, None] * W1[:, w][None, :]

    w2blk = np.zeros((128, 48))
    for q in range(4):
        w2blk[32 * q:32 * q + 32, 12 * q:12 * q + 12] = W2.T

    # Matmul operand base partitions are restricted to {0,32,64}; use
    # block-diagonal masked lhsT's with K=128 and full-tile outputs instead
    # of partition-sliced operands.  Constants ship as two DMAs: blob1 holds
    # everything the early PE queue waits on (unblocks the depth-4 PE wait
    # queue before the r transposes), blob2 the late-stage matmul weights.
    blob1 = np.zeros((128, CB1), np.float16)
    blob1[:, 0:128] = np.eye(128, dtype=np.float16)
    for g in range(4):
        blob1[32 * g:32 * g + 32, 128 + 128 * g:128 + 128 * g + 128] = \
            U4.astype(np.float16)
    blob1[0, 640:768] = np.tile(b1, 4).astype(np.float16)
    blob1[0, 768:816] = np.tile(b2, 4).astype(np.float16)
    blob2 = np.zeros((128, CB2), np.float16)
    for g in range(4):
        blob2[:, 128 * g + 32 * g:128 * g + 32 * g + 32] = \
            SW.astype(np.float16)
    blob2[:, 512:560] = w2blk.astype(np.float16)
    return dict(cblob1=np.ascontiguousarray(blob1),
                cblob2=np.ascontiguousarray(blob2))


# ----------------------------------------------------------------------------
# Device kernel body (Bass/Tile)
# ----------------------------------------------------------------------------


def _build_body(ctx, tc, x, cblob1, cblob2, y):
    import concourse.bass as bass
    from concourse import mybir
    nc = tc.nc
    f32 = mybir.dt.float32
    f16 = mybir.dt.float16
    AF = mybir.ActivationFunctionType

    def fv(t, col, dims):
        """View of tile t at free-offset col with custom free dims."""
        return bass.AP(tensor=t.tensor, offset=t.offset + col,
                       ap=[list(t.ap[0])] + [list(d) for d in dims])

    consts = ctx.enter_context(tc.tile_pool(name="consts", bufs=1))
    sb = ctx.enter_context(tc.tile_pool(name="sb", bufs=1))
    ps = ctx.enter_context(tc.tile_pool(name="ps", bufs=1, space="PSUM"))

    # ---- DMAs: x first (critical), then consts (critical part first)
    x_s = sb.tile([128, 96], f32)
    xa = bass.AP(tensor=x.tensor, offset=0, ap=[[96, 128], [1, 96]])
    nc.sync.dma_start(x_s[:, :], xa)
    c1 = consts.tile([128, CB1], f16)
    nc.sync.dma_start(c1[:, :], cblob1)
    c2t = consts.tile([128, CB2], f16)
    nc.sync.dma_start(c2t[:, :], cblob2)

    ident = c1[:, 0:128]
    U4blk = c1[:, 128:640]
    b1c = c1[0:1, 640:768]
    b2r = c1[0:1, 768:816]
    SWblk = c2t[:, 0:512]
    w2 = c2t[:, 512:560]

    ones1 = consts.tile([1, 128], f16)
    nc.vector.memset(ones1[:, :], 1.0)
    bias_s = consts.tile([128, 1], f32)
    nc.vector.memset(bias_s[:, :], 0.0)
    bias_c = consts.tile([128, 1], f32)
    nc.vector.memset(bias_c[:, :], math.pi / 2)
    # warm the ACT Sin table while DMAs run (cold table load ~1.3us)
    warm = consts.tile([128, 1], f16)
    nc.scalar.activation(warm[:, :], bias_s[:, 0:1], AF.Sin,
                         bias=bias_s[:, 0:1], scale=1.0)

    # ---- sin/cos of x/2 for wires 0..4 of all 8 groups (fp16)
    # cs[p, 10q + j] = cos(x/2), cs[p, 10q + 5 + j] = sin(x/2)
    # cos(x/2) = Sin(x*0.5 + pi/2): table arg reaches ~3.9 for the extreme
    # N(0,1) tail where the Sin table is ~2e-3 accurate — inside tolerance.
    cs = sb.tile([128, 80], f16)
    xin = fv(x_s, 0, [[12, 8], [1, 5]])
    nc.scalar.activation(fv(cs, 0, [[10, 8], [1, 5]]), xin, AF.Sin,
                         bias=bias_c[:, 0:1], scale=0.5)
    nc.scalar.activation(fv(cs, 5, [[10, 8], [1, 5]]), xin, AF.Sin,
                         bias=bias_s[:, 0:1], scale=0.5)

    # ---- kron tree per half: h=0 on DVE, h=1 on GPSIMD (parallel)
    rh = []
    for h in (0, 1):
        E = {"dve": nc.vector, "gpsimd": nc.gpsimd}[CFG["kron"][h]]
        A = sb.tile([128, 16], f16, name=f"A{h}")
        C = sb.tile([128, 16], f16, name=f"C{h}")
        Bt = sb.tile([128, 32], f16, name=f"B{h}")
        rt = sb.tile([128, 128], f16, name=f"r{h}")
        o = 40 * h
        E.tensor_mul(fv(A, 0, [[4, 4], [2, 2], [1, 2]]),
                     fv(cs, o + 1, [[10, 4], [0, 2], [5, 2]]),
                     fv(cs, o + 0, [[10, 4], [5, 2], [0, 2]]))
        E.tensor_mul(fv(C, 0, [[4, 4], [2, 2], [1, 2]]),
                     fv(cs, o + 4, [[10, 4], [0, 2], [5, 2]]),
                     fv(cs, o + 3, [[10, 4], [5, 2], [0, 2]]))
        E.tensor_mul(fv(Bt, 0, [[8, 4], [2, 4], [1, 2]]),
                     fv(cs, o + 2, [[10, 4], [0, 4], [5, 2]]),
                     fv(A, 0, [[4, 4], [1, 4], [0, 2]]))
        E.tensor_mul(fv(rt, 0, [[32, 4], [4, 8], [1, 4]]),
                     fv(Bt, 0, [[8, 4], [1, 8], [0, 4]]),
                     fv(C, 0, [[4, 4], [0, 8], [1, 4]]))
        rh.append(rt)

    # ---- per half: transpose, G = U4.T @ rT_g, P = G^2, hT = SW.T @ P + b1,
    #      y4 = relu(hT).T @ w2blk + b2
    # Engine assignments are config-driven (see CFG) — the tile scheduler's
    # ordering interacts with them in hard-to-predict ways, so the winning
    # combination was found by sweeping TimelineSim.
    def _eng(name):
        return {"dve": nc.vector, "act": nc.scalar}[name]

    Gp_t = [ps.tile([128, 512], f32, name=f"Gp{h}") for h in (0, 1)]
    hTp_t = [ps.tile([128, 128], f32, name=f"hTp{h}") for h in (0, 1)]

    # optional PE p-state warmers: dependency-free matmuls over rotating
    # banks keep the PE busy through the DMA wait so real matmuls run at
    # full clock (pstate ramps after 3us of continuous PE activity); they
    # reuse the real G/hT banks, whose first real matmul is start=True.
    if CFG["pe_fillers"]:
        fsrc = consts.tile([128, 256], f16)
        nc.vector.memset(fsrc[:, :], 0.0)
        fbanks = [Gp_t[0][:, 0:256], Gp_t[1][:, 0:256],
                  hTp_t[0][:, 0:128], hTp_t[1][:, 0:128]]
        for i in range(CFG["pe_fillers"]):
            b = fbanks[i % 4]
            nc.tensor.matmul(b, lhsT=fsrc[:, 0:128],
                             rhs=fsrc[:, 0:b.free_size()],
                             start=True, stop=True)

    tr, cp, Ps_t = [], [], []
    for h in (0, 1):
        rT_p = ps.tile([128, 128], f16, name=f"rTp{h}")
        tr.append(nc.tensor.transpose(rT_p[:, :], rh[h][:, :], ident))
        rT_s = sb.tile([128, 128], f16, name=f"rTs{h}")
        E = _eng(CFG["rt_copy"][h])
        if E is nc.scalar:
            cp.append(nc.scalar.copy(rT_s[:, :], rT_p[:, :]))
        else:
            cp.append(nc.vector.tensor_copy(rT_s[:, :], rT_p[:, :]))
        G_p = Gp_t[h]
        for g in range(4):
            nc.tensor.matmul(G_p[:, 128 * g:128 * g + 128],
                             lhsT=U4blk[:, 128 * g:128 * g + 128],
                             rhs=rT_s[:, :], start=True, stop=True)
        Ps_t.append(sb.tile([128, 512], f16, name=f"Ps{h}"))

    from concourse.bass import _add_dep_helper
    if CFG["tr_edge"]:
        _add_dep_helper(tr[1].ins, tr[0].ins, sync=False, reason="t0 first")

    # squares, per (half, 256-col chunk): "act" = ACT Square from PSUM;
    # "dve" = DVE copy PSUM->SBUF f16 then fp16 2x self-multiply
    sq_ins = {}
    for h in (0, 1):
        for c in (0, 1):
            sl = slice(256 * c, 256 * c + 256)
            if CFG["sq"][h][c] == "act":
                sq_ins[(h, c)] = nc.scalar.activation(
                    Ps_t[h][:, sl], Gp_t[h][:, sl], AF.Square)
            else:
                Gc_s = sb.tile([128, 256], f16, name=f"Gc{h}{c}")
                nc.vector.tensor_copy(Gc_s[:, :], Gp_t[h][:, sl])
                nc.vector.tensor_mul(Ps_t[h][:, sl], Gc_s[:, :], Gc_s[:, :])
    if CFG["sq_edge"] and (0, 0) in sq_ins and (1, 0) in sq_ins:
        _add_dep_helper(sq_ins[(1, 0)].ins, sq_ins[(0, 0)].ins, sync=False,
                        reason="sq h0A first")

    for h in (0, 1):
        P_s = Ps_t[h]
        hT_p = hTp_t[h]
        nc.tensor.matmul(hT_p[:, :], lhsT=b1c, rhs=ones1[:, :],
                         start=True, stop=False)
        for g in range(4):
            nc.tensor.matmul(hT_p[:, :],
                             lhsT=SWblk[:, 128 * g:128 * g + 128],
                             rhs=P_s[:, 128 * g:128 * g + 128],
                             start=False, stop=(g == 3))
        hT_s = sb.tile([128, 128], f16, name=f"hTs{h}")
        if CFG["relu"][h] == "act":
            nc.scalar.activation(hT_s[:, :], hT_p[:, :], AF.Relu)
        else:
            nc.vector.tensor_scalar_max(hT_s[:, :], hT_p[:, :], 0.0)

        y4_p = ps.tile([128, 48], f32, name=f"y4p{h}")
        nc.tensor.matmul(y4_p[:, :], lhsT=ones1[:, :], rhs=b2r,
                         start=True, stop=False)
        nc.tensor.matmul(y4_p[:, :], lhsT=hT_s[:, :], rhs=w2,
                         start=False, stop=True)
        if CFG["split_out"]:
            y_sh = sb.tile([128, 48], f32, name=f"ysh{h}")
            if CFG["y_copy"][h] == "act":
                nc.scalar.copy(y_sh[:, :], y4_p[:, :])
            else:
                nc.vector.tensor_copy(y_sh[:, :], y4_p[:, :])
            ya = bass.AP(tensor=y.tensor, offset=48 * h,
                         ap=[[96, 128], [1, 48]])
            nc.sync.dma_start(ya, y_sh[:, :])
        else:
            if "y_s" not in locals():
                y_s = sb.tile([128, 96], f32)
            if CFG["y_copy"][h] == "act":
                nc.scalar.copy(fv(y_s, 48 * h, [[12, 4], [1, 12]]),
                               fv(y4_p, 0, [[12, 4], [1, 12]]))
            else:
                nc.vector.tensor_copy(fv(y_s, 48 * h, [[12, 4], [1, 12]]),
                                      fv(y4_p, 0, [[12, 4], [1, 12]]))
    if not CFG["split_out"]:
        ya = bass.AP(tensor=y.tensor, offset=0, ap=[[96, 128], [1, 96]])
        nc.sync.dma_start(ya, y_s[:, :])


_NC_CACHE = {}


def _get_nc():
    if "nc" in _NC_CACHE:
        return _NC_CACHE["nc"]
    from contextlib import ExitStack
    import concourse.bacc as bacc
    import concourse.tile as tile
    from concourse import mybir
    f32 = mybir.dt.float32
    f16 = mybir.dt.float16
    nc = bacc.Bacc("TRN2", target_bir_lowering=False, debug=False)
    x = nc.dram_tensor("x", [BLOC, 12], f32, kind="ExternalInput").ap()
    cblob1 = nc.dram_tensor("cblob1", [128, CB1], f16,
                            kind="ExternalInput").ap()
    cblob2 = nc.dram_tensor("cblob2", [128, CB2], f16,
                            kind="ExternalInput").ap()
    y = nc.dram_tensor("y", [BLOC, 12], f32, kind="ExternalOutput").ap()
    with tile.TileContext(nc) as tc:
        with ExitStack() as ctx:
            _build_body(ctx, tc, x, cblob1, cblob2, y)
    nc.compile()
    _NC_CACHE["nc"] = nc
    return nc


def _run(inputs_np, consts, trace=False):
    from concourse.bass_utils import run_bass_kernel_spmd
    nc = _get_nc()
    x = np.ascontiguousarray(np.asarray(inputs_np, np.float32))
    in_maps = []
    for c in range(NCORES):
        m = {"x": np.ascontiguousarray(x[BLOC * c:BLOC * (c + 1)])}
        m.update(consts)
        in_maps.append(m)
    res = run_bass_kernel_spmd(nc, in_maps, core_ids=list(range(NCORES)),
                               trace=trace)
    out = np.concatenate([r["y"] for r in res.results], axis=0)
    return out.astype(np.float32), res


def kernel(inputs, q_params, W1, b1, W2, b2):
    consts = _host_consts(q_params, W1, b1, W2, b2)
    out, _ = _run(inputs, consts, trace=False)
    return out
